# revision 31
# baseline (speedup 1.0000x reference)
"""Batched GNN neighbor aggregation on 8 NeuronCores.

out[b] = neibors[b] @ last_embs[b]  for b in 0..7  (2048x2048 @ 2048x128, f32)

Sharding: one graph per core (batch dim across the 8 cores), no cross-core
communication. The device computes out^T = embs^T @ neibors^T with the
embedding chunks stationary; the host transposes the small result back.

Precision scheme (measured max-rel error 1.955013e-2 on the reference
inputs, deterministic run-to-run; gate 2e-2):
- 7 k-chunks in fp16 (2B/elem), E in fp16, one 1-cycle/row pass each.
- 8 k-chunks in fp8e4m3 (1B/elem) as 4 DoubleRow pairs. E's fp8 error is
  fixed with a second weights pass: E8hi = fp8(E) and E8lo =
  fp8(E - fp8(E)) (tiny values, stored unscaled) both matmul the SAME
  fp8 A data in SBUF into the same f32 PSUM group - no extra A traffic.
- chunk 6 in fp8 as a single DoubleRow pass: stationary (E8hi, E8lo),
  moving = the SAME fp8 A chunk read twice via a step-0 broadcast AP,
  giving 16-bit-E precision at fp16-pass cycle cost and fp8 traffic.
Stream: 5.75 MB A + 0.5 MB E + 0.5 MB out(fp16) per core.

Schedule (from trace analysis of the previous version):
- All HWDGE DMAs issued on one engine serialize through ONE hardware FIFO
  ring served by all 16 SDMA engines at ~410 GB/s aggregate; transfers
  complete strictly in issue order. So: every transfer (E payload first,
  then A) is issued on sync in exact consumption order and the data
  stream itself paces the PE with no out-of-order hazards.
- 5.75 MB A + 0.5 MB E at ~370-410 GB/s ≈ 16 us of stream time; PE
  needs ~14.5 us warm (64 x N=512-col matmul groups). The kernel is
  DMA-stream-bound; the PE must simply never go cold.
- fp8 DR pairs are processed FIRST: they need ~1.9 us of PE per 512 KB
  vs 0.86 us for fp16 chunks, so the PE builds backlog while the stream
  ramps and the fp16 chunks ride the tail where data is already ahead.
- Prewarm matmuls on an *uninitialized* scratch tile (no memset, no
  deps) start the instant the engine preamble ends and bridge the
  ~3.5 us until pair 0 lands, holding the HAM clock gate at full rate
  (idle >3.4 us re-throttles the PE to half clock).
- The final fp16 chunk arrives as two half-transfers and its bank
  matmuls run in reverse bank order, so each PSUM bank closes (copy +
  store, alternating engines) as soon as possible and the last-closing
  bank has the shortest copy->store chain.
"""

import numpy as np
import ml_dtypes

FP8 = ml_dtypes.float8_e4m3

B = 8
N = 2048
D = 128
KT = 128
NT = 512
NK = 16        # k-chunks total
NP8 = 4        # fp8 DoubleRow pairs (cover chunks 7..14)
NF16 = 7       # fp16 chunks: indices 0..5 and 15
# chunk 6 streams as a single fp8 chunk, processed in ONE DoubleRow pass
# with stationary (E8hi, E8lo) and the SAME A data read twice through a
# step-0 broadcast AP: result = (E8hi + E8lo).T @ A, i.e. 16-bit E
# precision at fp16-pass cycle cost with fp8 A traffic.
NN = N // NT   # 4
NWARM = 46     # prewarm matmuls (N=128 each) bridging preamble -> first data

_cached_nc = None


def _dedup_ldweights(nc, mybir):
    """Drop InstLdweights whose weight AP matches the immediately preceding
    weight load in the PE stream (matmuls here have ldweights=False, so the
    stationary operand stays in the array between identical loads)."""
    for bb in nc.m.functions[0].blocks:
        insts = bb.instructions
        last_key = None
        removed = []
        for inst in insts:
            if getattr(inst, "engine", None) != mybir.EngineType.PE:
                continue
            ty = type(inst).__name__
            if ty == "InstLdweights":
                key = repr(inst.ins[0])
                if key == last_key and not inst.has_wait():
                    removed.append(inst)
                else:
                    last_key = key
            elif ty != "InstMatmult":
                last_key = None
        if removed:
            rm = {id(i) for i in removed}
            insts[:] = [i for i in insts if id(i) not in rm]
            for i in removed:
                nc.inst_map.pop(i.name, None)


def _build_program():
    import concourse.tile as tile
    from concourse import bacc, mybir

    f32 = mybir.dt.float32
    fp16 = mybir.dt.float16
    fp8 = mybir.dt.float8e4
    DR = mybir.MatmulPerfMode.DoubleRow
    nc = bacc.Bacc(
        "TRN2",
        target_bir_lowering=False,
        debug=False,
        enable_asserts=False,
        enable_partition_id=False,
    )

    # a16[i]: fp16 chunks in PROCESSING order (processed after the pairs)
    a16 = nc.dram_tensor("a16", [NF16, KT, N], fp16, kind="ExternalInput")
    # a8[j][p, n, i] = fp8 DR pair j (chunks interleaved on the last axis)
    a8 = nc.dram_tensor("a8", [NP8, KT, N, 2], fp8, kind="ExternalInput")
    # a6: chunk 6 as a plain fp8 chunk (broadcast-DR pass)
    a6 = nc.dram_tensor("a6", [KT, N], fp8, kind="ExternalInput")
    # eall[p, :2048] = e8 bytes [s, j, i, d] (s=0 hi, s=1 lo residual);
    # eall[p, 2048:2304] = chunk-6 (E8hi, E8lo) bytes [s, d];
    # eall[p, 2304:] = e16 bytes [i, d] as fp16. One packed tensor so the
    # whole E payload moves as a single 4KB-per-partition-line transfer
    # (2KB lines halve DMA throughput; 1KB lines are worse).
    eall = nc.dram_tensor("eall", [KT, 4096], fp8, kind="ExternalInput")
    out_t = nc.dram_tensor("out_t", [D, N], fp16, kind="ExternalOutput")

    with tile.TileContext(nc) as tc:
        with (
            tc.tile_pool(name="econst", bufs=1) as epool,
            tc.tile_pool(name="ahi", bufs=12) as hpool,
            tc.tile_pool(name="psum", bufs=1, space="PSUM") as pspool,
            tc.tile_pool(name="out", bufs=1) as opool,
        ):
            # HAM prewarm: scratch matmuls on an uninitialized tile (the
            # numeric garbage is discarded) so the PE starts the moment the
            # preamble barrier drops and is at full clock when pair 0 lands.
            wu = epool.tile([KT, KT], fp16, name="wu")
            wu_ps = pspool.tile([KT, KT], f32, name="wups", tag="wups")
            nc.vector.memset(wu[:], 0.0)
            for _ in range(NWARM):
                nc.tensor.matmul(wu_ps[:], wu[:], wu[:], start=True, stop=True)

            eall_sb = epool.tile([KT, 4096], fp8, name="eall_sb")
            e8_sb = eall_sb[:, 0:2048].rearrange(
                "p (s j i d) -> p s j i d", s=2, j=NP8, i=2, d=D
            )
            es6 = eall_sb[:, 2048:2304].rearrange(
                "p (s d) -> p s d", s=2, d=D
            )
            e_sb = (
                eall_sb[:, 2304:4096]
                .bitcast(fp16)
                .rearrange("p (i d) -> p i d", i=NF16, d=D)
            )

            his = [
                hpool.tile([KT, N], fp16, name=f"hi{i}", tag="hi")
                for i in range(NF16)
            ]
            prs = [
                hpool.tile([KT, N, 2], fp8, name=f"pr{j}", tag="hi")
                for j in range(NP8)
            ]
            a6_sb = hpool.tile([KT, N], fp8, name="a6s", tag="hi")

            # --- DMA issue schedule ---
            # Everything the PE consumes rides sync's single HWDGE FIFO in
            # exact consumption order; completions are strictly in-order so
            # each item's semaphore paces the PE with no cross-lane
            # hazards (a second queue is served at a fraction of sync's
            # rate once the A stream saturates, and cross-queue semaphore
            # lane reuse stalls the issue pipeline - keep it all on sync).
            # The final fp16 chunk is split in two half-transfers so the
            # last two PSUM banks close (and store) as soon as their half
            # lands - the stream tail is exposed to cross-core HBM
            # contention, so keep it small.
            nc.sync.dma_start(eall_sb[:], eall.ap())
            for j in range(NP8):
                nc.sync.dma_start(prs[j][:], a8.ap()[j])
            nc.sync.dma_start(a6_sb[:], a6.ap())
            for i in range(NF16 - 1):
                nc.sync.dma_start(his[i][:], a16.ap()[i])
            last = NF16 - 1
            H = N // 2
            nc.sync.dma_start(his[last][:, :H], a16.ap()[last][:, :H])
            nc.sync.dma_start(his[last][:, H:], a16.ap()[last][:, H:])

            ps = [
                pspool.tile([D, NT], f32, name=f"ps{n}", tag=f"ps{n}")
                for n in range(NN)
            ]

            # All fp8 pairs first (hi then lo pass per pair): a DR weight
            # load right after fp16 passes stalls ~400ns (it needs both
            # weight planes), so DR passes are grouped rather than
            # interleaved; the pairs' 2x PE-time per byte also buffers the
            # PE against the slow (~200-250 GB/s) head of the DMA stream.
            def _pair(j):
                pr = prs[j]
                for s in (0, 1):
                    for n in range(NN):
                        nc.tensor.matmul(
                            ps[n][:],
                            e8_sb[:, s, j, :, :],
                            pr[:, n * NT : (n + 1) * NT, :].transpose(
                                [0, 2, 1]
                            ),
                            start=(j == 0 and s == 0),
                            stop=False,
                            perf_mode=DR,
                        )

            def _chunk(i):
                hi = his[i]
                for n in range(NN):
                    nc.tensor.matmul(
                        ps[n][:],
                        e_sb[:, i, :],
                        hi[:, n * NT : (n + 1) * NT],
                        start=False,
                        stop=False,
                    )

            for j in range(NP8):
                _pair(j)
            # chunk 6: one DR pass, weights (E8hi, E8lo), A read twice
            for n in range(NN):
                nc.tensor.matmul(
                    ps[n][:],
                    es6,
                    a6_sb[:, n * NT : (n + 1) * NT]
                    .unsqueeze(1)
                    .broadcast_to([KT, 2, NT]),
                    start=False,
                    stop=False,
                    perf_mode=DR,
                )
            for i in range(NF16 - 1):
                _chunk(i)

            # Finale: each bank's last matmul (final fp16 chunk) closes it
            # and is immediately followed by its PSUM copy + store; copies
            # and stores alternate engines so the four drains overlap.
            def _close(n):
                o_sb = opool.tile([D, NT], fp16, name=f"o{n}", tag=f"o{n}")
                if n % 2 == 0:
                    nc.vector.tensor_copy(o_sb[:], ps[n][:])
                else:
                    nc.scalar.copy(o_sb[:], ps[n][:])
                (nc.sync if n % 2 == 0 else nc.scalar).dma_start(
                    out_t.ap()[:, n * NT : (n + 1) * NT], o_sb[:]
                )

            for n in reversed(range(NN)):
                nc.tensor.matmul(
                    ps[n][:],
                    e_sb[:, last, :],
                    his[last][:, n * NT : (n + 1) * NT],
                    start=False,
                    stop=True,
                )
                _close(n)

    try:
        _dedup_ldweights(nc, mybir)
    except Exception:
        pass
    nc.compile()
    return nc


def _make_in_maps(last_embs, neibors):
    in_maps = []
    # fp16 chunks [0..5, 15]; single-fp8 chunk 6; fp8 pair chunks 7..14
    f16_idx = list(range(6)) + [NK - 1]
    for g in range(B):
        at = np.ascontiguousarray(neibors[g].T)  # [m, n] f32
        atc = at.reshape(NK, KT, N)
        a16_g = atc[f16_idx].astype(np.float16)
        a6_g = atc[6].astype(FP8)
        a8_g = (
            atc[7 : NK - 1]
            .astype(FP8)
            .reshape(NP8, 2, KT, N)
            .transpose(0, 2, 3, 1)
        )
        eg = last_embs[g].reshape(NK, KT, D)
        e16_g = eg[f16_idx].astype(np.float16).transpose(1, 0, 2)
        e8t = eg[7 : NK - 1]  # [2*NP8, KT, D]
        e8h = e8t.astype(FP8)
        e8l = (e8t - e8h.astype(np.float32)).astype(FP8)
        # [2, NP8, 2, KT, D] -> [KT, 2, NP8, 2, D]
        e8_g = np.stack(
            [e8h.reshape(NP8, 2, KT, D), e8l.reshape(NP8, 2, KT, D)], axis=0
        ).transpose(3, 0, 1, 2, 4)
        # chunk-6 (E8hi, E8lo): [KT, 2, D]
        e6h = eg[6].astype(FP8)
        e6l = (eg[6] - e6h.astype(np.float32)).astype(FP8)
        es6_g = np.stack([e6h, e6l], axis=1)
        # pack per partition: 2048 e8 | 256 es6 | 1792 e16 bytes
        eall_g = np.concatenate(
            [
                np.ascontiguousarray(e8_g).view(np.uint8).reshape(KT, 2048),
                np.ascontiguousarray(es6_g).view(np.uint8).reshape(KT, 256),
                np.ascontiguousarray(e16_g).view(np.uint8).reshape(KT, 1792),
            ],
            axis=1,
        )
        in_maps.append(
            {
                "a16": np.ascontiguousarray(a16_g),
                "a8": np.ascontiguousarray(a8_g),
                "a6": np.ascontiguousarray(a6_g),
                "eall": np.ascontiguousarray(eall_g).view(FP8),
            }
        )
    return in_maps


def kernel(last_embs, neibors):
    global _cached_nc
    from concourse.bass_utils import run_bass_kernel_spmd

    last_embs = np.asarray(last_embs, dtype=np.float32)
    neibors = np.asarray(neibors, dtype=np.float32)
    if _cached_nc is None:
        _cached_nc = _build_program()
    in_maps = _make_in_maps(last_embs, neibors)
    try:
        res = run_bass_kernel_spmd(_cached_nc, in_maps, list(range(B))).results
    except Exception:
        # transient NRT/terminal hiccups have been observed; retry once
        import time

        time.sleep(15)
        res = run_bass_kernel_spmd(_cached_nc, in_maps, list(range(B))).results
    out = np.stack(
        [res[g]["out_t"].T.astype(np.float32) for g in range(B)], axis=0
    )
    return np.ascontiguousarray(out)
